# revision 19
# baseline (speedup 1.0000x reference)
"""Trainium2 Bass kernel for the BaselineEngine cell-swarm module.

Sharding: data-parallel over the cell axis across 8 NeuronCores; 32768
cells per core = exactly one faction per core, so faction means are
core-local and only a tiny packed AllReduce ([256] floats: faction sum,
softmax numerator S, softmax denominator Z, tension sum) crosses cores.

Per-core layout is feature-major ([feature, cell]) so that the MLP and
GRU matmuls chain on the TensorEngine without transposes.  The single
shared input x is folded into the first-layer biases on the host.
Sigmoid is computed as (1 + tanh(x/2))/2 so every transcendental lives
in the one ACT table set; the resulting affine terms are folded into
the PE-side accumulations and the GRU combine:

    new_h_pre = nctl + z*(h - nctl),  z = (1+th_z)/2
    w := 2*new_h_pre = (th_z+1)*h - (th_z-1)*nctl

w is kept resident in SBUF; after a local faction-sum reduce and the
AllReduce, phase 2 blends w with the faction/global means and streams
the result out.
"""

import sys
import types

sys.path.insert(0, "/opt/trn_rl_repo")

import numpy as np
import ml_dtypes

import concourse.bass as bass
import concourse.bacc as bacc
import concourse.tile as tile
from concourse import mybir
from concourse.bass_utils import run_bass_kernel_spmd

BF16 = mybir.dt.bfloat16
F32 = mybir.dt.float32
AL = mybir.AluOpType
AF = mybir.ActivationFunctionType

N_CORES = 8
HID = 128
OUT = 64
IN_DIM = 64
FD_G = 2048          # cells per group (SBUF-op granularity)
FD_C = 1024          # cells per PSUM chunk
MM_N = 512           # matmul moving free dim


def _install_ntff_hook():
    """Make antenv.axon_hooks importable so trace=True works here."""
    if "antenv.axon_hooks" in sys.modules:
        return
    try:
        import antenv
        from trn_agent_boot.trn_boot import _ntff_profile_via_ctypes
    except Exception:
        return
    mod = types.ModuleType("antenv.axon_hooks")
    mod._hook = None

    def set_axon_ntff_profile_hook(h):
        mod._hook = h

    def get_axon_ntff_profile_hook():
        return mod._hook

    mod.set_axon_ntff_profile_hook = set_axon_ntff_profile_hook
    mod.get_axon_ntff_profile_hook = get_axon_ntff_profile_hook
    sys.modules["antenv.axon_hooks"] = mod
    antenv.axon_hooks = mod
    try:
        h = _ntff_profile_via_ctypes("/opt/axon/libaxon_pjrt.so")
        if h is not None:
            set_axon_ntff_profile_hook(h)
    except Exception:
        pass


def build_kernel(n_loc: int, n_cores: int = N_CORES):
    """Emit the per-core Bass program for n_loc cells per core."""
    n_groups = n_loc // FD_G
    n_chunks = n_loc // FD_C
    # debate rows = first fs//4 cells of the faction; group-aligned by design
    dc = n_loc // 4
    assert dc % FD_G == 0
    dbg_groups = dc // FD_G

    nc = bacc.Bacc(None, num_devices=n_cores)

    hT_in = nc.declare_dram_parameter("hT_in", [HID, n_loc], BF16, False)
    waT_in = nc.declare_dram_parameter("waT_in", [HID, HID], BF16, False)
    wgT_in = nc.declare_dram_parameter("wgT_in", [HID, HID], BF16, False)
    w2T_in = nc.declare_dram_parameter("w2T_in", [HID, HID], BF16, False)
    wihT_in = nc.declare_dram_parameter("wihT_in", [OUT + 2, 3 * HID], BF16, False)
    whhT_in = nc.declare_dram_parameter("whhT_in", [HID, 2 * HID], BF16, False)
    whhTn_in = nc.declare_dram_parameter("whhTn_in", [HID, HID], BF16, False)
    cst_in = nc.declare_dram_parameter("cst_in", [HID, 8], F32, False)
    nhT_out = nc.declare_dram_parameter("nhT_out", [HID, n_loc], BF16, True)
    par_out = nc.declare_dram_parameter("par_out", [256, 1], F32, True)

    with tile.TileContext(nc) as tc:
        with (
            tc.tile_pool(name="const", bufs=1) as constp,
            tc.tile_pool(name="pre", bufs=1) as prep,
            tc.tile_pool(name="acc", bufs=1) as accp,
            tc.tile_pool(name="hin", bufs=4) as hinp,
            tc.tile_pool(name="work", bufs=2) as workp,
            tc.tile_pool(name="late", bufs=2) as latep,
            tc.tile_pool(name="psf", bufs=2, space="PSUM") as psfp,
            tc.tile_pool(name="psm", bufs=2, space="PSUM") as psmp,
            tc.tile_pool(name="dram", bufs=1, space="DRAM") as dramp,
        ):
            # ---- constants ----
            waT = constp.tile([HID, HID], BF16)
            wgT = constp.tile([HID, HID], BF16)
            w2T = constp.tile([HID, HID], BF16)       # [:,0:64]=Wa2T, [:,64:128]=-Wg2T
            wihT = constp.tile([OUT + 2, 3 * HID], BF16)
            whhT = constp.tile([HID, 2 * HID], BF16)
            whhTn = constp.tile([HID, HID], BF16)     # 0.5*Whh_n^T
            cst = constp.tile([HID, 8], F32)
            ones65 = constp.tile([OUT, OUT + 1], BF16)
            ones_row = constp.tile([1, FD_G], BF16)
            nc.vector.memset(ones_row[:], 1.0)
            nc.gpsimd.dma_start(out=waT[:], in_=waT_in[:])
            nc.gpsimd.dma_start(out=wgT[:], in_=wgT_in[:])
            nc.gpsimd.dma_start(out=w2T[:], in_=w2T_in[:])
            nc.gpsimd.dma_start(out=wihT[:], in_=wihT_in[:])
            nc.gpsimd.dma_start(out=whhT[:], in_=whhT_in[:])
            nc.gpsimd.dma_start(out=whhTn[:], in_=whhTn_in[:])
            nc.gpsimd.dma_start(out=cst[:], in_=cst_in[:])
            nc.vector.memset(ones65[:], 1.0)
            abias = cst[:, 0:1]
            gbias = cst[:, 1:2]
            dba = cst[0:OUT, 2:3]
            bhhn_half = cst[:, 3:4]
            dalpha = cst[:, 4:5]
            dc1 = cst[:, 5:6]
            dc2 = cst[:, 6:7]

            # ---- persistent state ----
            pre = prep.tile([HID, n_loc], BF16)            # w = 2*new_h_pre
            wsum = accp.tile([HID, n_groups], F32)         # per-group sum of w
            s_cols = accp.tile([OUT, n_chunks], F32)       # softmax numerator partials
            z_cols = accp.tile([HID, n_chunks], F32)       # rows 0:65 valid
            t_cols = accp.tile([HID, n_chunks], F32)       # row 64 valid

            # =================== phase 1 (software-pipelined) ===================
            NCC = FD_G // FD_C
            tiles = {}

            def front(g):
                c0 = g * FD_G
                hT = hinp.tile([HID, FD_G], BF16, tag="hT")
                nc.sync.dma_start(out=hT[:], in_=hT_in[:, c0:c0 + FD_G])
                mem = workp.tile([OUT + 2, FD_G], BF16, tag="mem")
                nc.sync.dma_start(out=mem[OUT + 1:OUT + 2, :], in_=ones_row[:])
                h1a = workp.tile([HID, FD_G], BF16, tag="h1a")
                h1g = workp.tile([HID, FD_G], BF16, tag="h1g")
                out2 = workp.tile([OUT, FD_G], BF16, tag="o2w")
                wexp = workp.tile([OUT + 1, FD_G], BF16, tag="wexp")
                wsc = workp.tile([OUT, FD_G], BF16, tag="o2w")
                tiles[g] = dict(hT=hT, mem=mem)
                for cc in range(NCC):
                    sl = slice(cc * FD_C, (cc + 1) * FD_C)
                    ci = g * NCC + cc
                    psA = psfp.tile([HID, FD_C], F32, tag="psf")
                    for q in range(FD_C // MM_N):
                        qs = slice(cc * FD_C + q * MM_N, cc * FD_C + (q + 1) * MM_N)
                        qd = slice(q * MM_N, (q + 1) * MM_N)
                        nc.tensor.matmul(psA[:, qd], waT[:], hT[:, qs], start=True, stop=True)
                    nc.scalar.activation(out=h1a[:, sl], in_=psA[:], func=AF.Relu,
                                         bias=abias, scale=1.0)
                    psG = psfp.tile([HID, FD_C], F32, tag="psf")
                    for q in range(FD_C // MM_N):
                        qs = slice(cc * FD_C + q * MM_N, cc * FD_C + (q + 1) * MM_N)
                        qd = slice(q * MM_N, (q + 1) * MM_N)
                        nc.tensor.matmul(psG[:, qd], wgT[:], hT[:, qs], start=True, stop=True)
                    nc.scalar.activation(out=h1g[:, sl], in_=psG[:], func=AF.Relu,
                                         bias=gbias, scale=1.0)
                    psO = psfp.tile([HID, FD_C], F32, tag="psf")
                    for q in range(FD_C // MM_N):
                        qs = slice(cc * FD_C + q * MM_N, cc * FD_C + (q + 1) * MM_N)
                        qd = slice(q * MM_N, (q + 1) * MM_N)
                        nc.tensor.matmul(psO[0:OUT, qd], w2T[:, 0:OUT], h1a[:, qs], start=True, stop=False)
                        nc.tensor.matmul(psO[0:OUT, qd], w2T[:, OUT:HID], h1g[:, qs], start=False, stop=True)
                    if cc % 2 == 0:
                        nc.scalar.activation(out=mem[0:OUT, sl], in_=psO[0:OUT, :],
                                             func=AF.Identity, bias=dba, scale=1.0)
                    else:
                        nc.vector.tensor_scalar(out=mem[0:OUT, sl], in0=psO[0:OUT, :],
                                                scalar1=dba, scalar2=None, op0=AL.add)
                    nc.vector.tensor_tensor(out=out2[:, sl], in0=mem[0:OUT, sl],
                                            in1=mem[0:OUT, sl], op=AL.mult)
                    psT = psfp.tile([HID, FD_C], F32, tag="psf")
                    for q in range(FD_C // MM_N):
                        qs = slice(cc * FD_C + q * MM_N, cc * FD_C + (q + 1) * MM_N)
                        qd = slice(q * MM_N, (q + 1) * MM_N)
                        nc.tensor.matmul(psT[0:OUT + 1, qd], ones65[:], out2[:, qs], start=True, stop=True)
                    nc.scalar.activation(out=wexp[:, sl], in_=psT[0:OUT + 1, :], func=AF.Exp,
                                         scale=1.0 / OUT, accum_out=z_cols[0:OUT + 1, ci:ci + 1])
                    nc.scalar.activation(out=mem[OUT:OUT + 1, sl], in_=psT[OUT:OUT + 1, :],
                                         func=AF.Copy, scale=1.0 / OUT,
                                         accum_out=t_cols[OUT:OUT + 1, ci:ci + 1])
                    nc.vector.scalar_tensor_tensor(
                        out=wsc[:, sl], in0=mem[0:OUT, sl], scalar=0.0, in1=wexp[0:OUT, sl],
                        op0=AL.bypass, op1=AL.mult, accum_out=s_cols[:, ci:ci + 1])

            def mid(g):
                d = tiles[g]
                hT, mem = d["hT"], d["mem"]
                thr = workp.tile([HID, FD_G], BF16, tag="thr")
                thz = workp.tile([HID, FD_G], BF16, tag="thz")
                mtl = latep.tile([HID, FD_G], BF16, tag="mtl")
                targ = latep.tile([HID, FD_G], BF16, tag="targ")
                d.update(thr=thr, thz=thz, targ=targ)
                for cc in range(NCC):
                    sl = slice(cc * FD_C, (cc + 1) * FD_C)
                    psR = psmp.tile([HID, FD_C], F32, tag="psm")
                    for q in range(FD_C // MM_N):
                        qs = slice(cc * FD_C + q * MM_N, cc * FD_C + (q + 1) * MM_N)
                        qd = slice(q * MM_N, (q + 1) * MM_N)
                        nc.tensor.matmul(psR[:, qd], wihT[:, 0:HID], mem[:, qs], start=True, stop=False)
                        nc.tensor.matmul(psR[:, qd], whhT[:, 0:HID], hT[:, qs], start=False, stop=True)
                    nc.scalar.activation(out=thr[:, sl], in_=psR[:], func=AF.Tanh, scale=0.5)
                    psZ = psmp.tile([HID, FD_C], F32, tag="psm")
                    for q in range(FD_C // MM_N):
                        qs = slice(cc * FD_C + q * MM_N, cc * FD_C + (q + 1) * MM_N)
                        qd = slice(q * MM_N, (q + 1) * MM_N)
                        nc.tensor.matmul(psZ[:, qd], wihT[:, HID:2 * HID], mem[:, qs], start=True, stop=False)
                        nc.tensor.matmul(psZ[:, qd], whhT[:, HID:2 * HID], hT[:, qs], start=False, stop=True)
                    nc.scalar.activation(out=thz[:, sl], in_=psZ[:], func=AF.Tanh, scale=0.5)
                    psA2 = psmp.tile([HID, FD_C], F32, tag="psm")
                    for q in range(FD_C // MM_N):
                        qs = slice(cc * FD_C + q * MM_N, cc * FD_C + (q + 1) * MM_N)
                        qd = slice(q * MM_N, (q + 1) * MM_N)
                        nc.tensor.matmul(psA2[:, qd], wihT[:, 2 * HID:3 * HID], mem[:, qs], start=True, stop=False)
                        nc.tensor.matmul(psA2[:, qd], whhTn[:], hT[:, qs], start=False, stop=True)
                    psB2 = psmp.tile([HID, FD_C], F32, tag="psm")
                    for q in range(FD_C // MM_N):
                        qs = slice(cc * FD_C + q * MM_N, cc * FD_C + (q + 1) * MM_N)
                        qd = slice(q * MM_N, (q + 1) * MM_N)
                        nc.tensor.matmul(psB2[:, qd], whhTn[:], hT[:, qs], start=True, stop=True)
                    nc.vector.scalar_tensor_tensor(
                        out=mtl[:, sl], in0=psB2[:], scalar=bhhn_half, in1=thr[:, sl],
                        op0=AL.add, op1=AL.mult)
                    nc.vector.tensor_tensor(out=targ[:, sl], in0=mtl[:, sl],
                                            in1=psA2[:], op=AL.add)

            def back(g):
                d = tiles.pop(g)
                c0 = g * FD_G
                nctl = latep.tile([HID, FD_G], BF16, tag="nctl")
                u1 = latep.tile([HID, FD_G], BF16, tag="u1")
                u2 = latep.tile([HID, FD_G], BF16, tag="mtl")
                nc.scalar.activation(out=nctl[:], in_=d["targ"][:], func=AF.Tanh)
                nc.vector.scalar_tensor_tensor(out=u1[:], in0=d["thz"][:], scalar=1.0,
                                               in1=d["hT"][:], op0=AL.add, op1=AL.mult)
                nc.vector.scalar_tensor_tensor(out=u2[:], in0=d["thz"][:], scalar=-1.0,
                                               in1=nctl[:], op0=AL.add, op1=AL.mult)
                nc.vector.scalar_tensor_tensor(
                    out=pre[:, c0:c0 + FD_G], in0=u1[:], scalar=0.0, in1=u2[:],
                    op0=AL.bypass, op1=AL.subtract, accum_out=wsum[:, g:g + 1])

            for t in range(n_groups + 2):
                if t < n_groups:
                    front(t)
                if 1 <= t <= n_groups:
                    mid(t - 1)
                if t >= 2:
                    back(t - 2)

            # =================== reductions + collective ===================
            fin = accp.tile([HID, 4], F32)
            nc.vector.tensor_reduce(out=fin[:, 0:1], in_=wsum[:], axis=mybir.AxisListType.X, op=AL.add)
            nc.vector.tensor_reduce(out=fin[0:OUT, 1:2], in_=s_cols[:], axis=mybir.AxisListType.X, op=AL.add)
            nc.vector.tensor_reduce(out=fin[0:1, 2:3], in_=z_cols[0:1, :], axis=mybir.AxisListType.X, op=AL.add)
            nc.vector.tensor_reduce(out=fin[OUT:OUT + 1, 3:4], in_=t_cols[OUT:OUT + 1, :],
                                    axis=mybir.AxisListType.X, op=AL.add)
            zpad = accp.tile([62, 1], F32)
            nc.vector.memset(zpad[:], 0.0)

            cc_in = dramp.tile([256, 1], F32)
            cc_out = dramp.tile([256, 1], F32)
            nc.sync.dma_start(out=cc_in[0:HID, :], in_=fin[:, 0:1])
            nc.sync.dma_start(out=cc_in[HID:HID + OUT, :], in_=fin[0:OUT, 1:2])
            nc.sync.dma_start(out=cc_in[192:193, :], in_=fin[0:1, 2:3])
            nc.sync.dma_start(out=cc_in[193:194, :], in_=fin[OUT:OUT + 1, 3:4])
            nc.sync.dma_start(out=cc_in[194:256, :], in_=zpad[:])
            nc.gpsimd.collective_compute(
                "AllReduce", AL.add, replica_groups=[list(range(n_cores))],
                ins=[cc_in.opt()], outs=[cc_out.opt()])
            nc.sync.dma_start(out=par_out[:], in_=cc_out[:])

            gsum = accp.tile([HID, 2], F32)
            nc.sync.dma_start(out=gsum[:, 0:1], in_=cc_out[0:HID, :])

            # =================== phase 2: blends ===================
            # non-debate: nh = 0.425*w + (0.075/n_loc)*fsum_w
            bv = accp.tile([HID, 2], F32)
            nc.vector.tensor_scalar(out=bv[:, 0:1], in0=fin[:, 0:1],
                                    scalar1=0.075 / n_loc, scalar2=None, op0=AL.mult)
            # debate: nh = dalpha*w + dc1*fsum_w + dc2*gsum_w
            nc.vector.tensor_scalar(out=gsum[:, 1:2], in0=fin[:, 0:1],
                                    scalar1=dc1, scalar2=None, op0=AL.mult)
            nc.vector.scalar_tensor_tensor(out=bv[:, 1:2], in0=gsum[:, 0:1],
                                           scalar=dc2, in1=gsum[:, 1:2],
                                           op0=AL.mult, op1=AL.add)

            dbg_p2 = dbg_groups
            order = list(range(dbg_p2, n_groups)) + list(range(dbg_p2))
            _tags = ["u1", "nctl", "mtl", "targ", "thr", "thz"]
            for i, g in enumerate(order):
                c0 = g * FD_G
                pool = latep if _tags[i % 6] not in ("thr", "thz") else workp
                st = pool.tile([HID, FD_G], BF16, tag=_tags[i % 6])
                if g < dbg_p2:
                    nc.vector.tensor_scalar(out=st[:], in0=pre[:, c0:c0 + FD_G],
                                            scalar1=dalpha, scalar2=bv[:, 1:2],
                                            op0=AL.mult, op1=AL.add)
                else:
                    nc.vector.tensor_scalar(out=st[:], in0=pre[:, c0:c0 + FD_G],
                                            scalar1=0.425, scalar2=bv[:, 0:1],
                                            op0=AL.mult, op1=AL.add)
                if i % 2 == 0:
                    nc.sync.dma_start(out=nhT_out[:, c0:c0 + FD_G], in_=st[:])
                else:
                    nc.gpsimd.dma_start(out=nhT_out[:, c0:c0 + FD_G], in_=st[:])

    nc.finalize()
    return nc


def make_host_consts(x, Wa1, ba1, Wa2, ba2, Wg1, bg1, Wg2, bg2,
                     Wih, Whh, bih, bhh, step, n_loc):
    """Fold x into biases; build packed bf16 weight layouts."""
    f32 = np.float32
    x0 = np.asarray(x, f32).reshape(-1)
    abias = np.asarray(ba1, f32) + np.asarray(Wa1, f32)[:, :IN_DIM] @ x0
    gbias = np.asarray(bg1, f32) + np.asarray(Wg1, f32)[:, :IN_DIM] @ x0
    dba = np.asarray(ba2, f32) - np.asarray(bg2, f32)

    bf = ml_dtypes.bfloat16
    waT = np.ascontiguousarray(np.asarray(Wa1, f32)[:, IN_DIM:].T).astype(bf)
    wgT = np.ascontiguousarray(np.asarray(Wg1, f32)[:, IN_DIM:].T).astype(bf)
    w2T = np.ascontiguousarray(
        np.concatenate([np.asarray(Wa2, f32).T, -np.asarray(Wg2, f32).T], axis=1)
    ).astype(bf)

    WihT = np.asarray(Wih, f32).T            # [65, 384]
    bih_ = np.asarray(bih, f32)
    bhh_ = np.asarray(bhh, f32)
    ones_row = np.concatenate([
        bih_[0:HID] + bhh_[0:HID],
        bih_[HID:2 * HID] + bhh_[HID:2 * HID],
        bih_[2 * HID:] + 0.5 * bhh_[2 * HID:],
    ])[None, :]
    wihT = np.ascontiguousarray(np.concatenate([WihT, ones_row], axis=0)).astype(bf)
    WhhT = np.asarray(Whh, f32).T
    whhT = np.ascontiguousarray(WhhT[:, 0:2 * HID]).astype(bf)
    whhTn = np.ascontiguousarray(0.5 * WhhT[:, 2 * HID:]).astype(bf)
    bhhn_half = 0.5 * bhh_[2 * HID:]

    # blend constants operate on w = 2*new_h_pre, hence the /2 factors
    if int(step) > 5:
        al2 = 0.85 * 0.85 / 2.0                 # debate rows: 0.7225*pre
        c1h = 0.85 * 0.15 / (2.0 * n_loc)       # 0.1275*fmean
        c2h = 0.15 / (2.0 * N_CORES * n_loc)    # 0.15*global mean
    else:
        al2 = 0.85 / 2.0
        c1h = 0.15 / (2.0 * n_loc)
        c2h = 0.0

    cst = np.zeros((HID, 8), f32)
    cst[:, 0] = abias
    cst[:, 1] = gbias
    cst[0:OUT, 2] = dba
    cst[:, 3] = bhhn_half
    cst[:, 4] = al2
    cst[:, 5] = c1h
    cst[:, 6] = c2h
    return dict(waT=waT, wgT=wgT, w2T=w2T, wihT=wihT, whhT=whhT,
                whhTn=whhTn, cst=cst)


def make_in_maps(hiddens, consts, n_loc):
    bf = ml_dtypes.bfloat16
    in_maps = []
    for c in range(N_CORES):
        shard = hiddens[c * n_loc:(c + 1) * n_loc]
        hT = np.ascontiguousarray(shard.T).astype(bf)
        in_maps.append({
            "hT_in": hT,
            "waT_in": consts["waT"], "wgT_in": consts["wgT"],
            "w2T_in": consts["w2T"], "wihT_in": consts["wihT"],
            "whhT_in": consts["whhT"], "whhTn_in": consts["whhTn"],
            "cst_in": consts["cst"],
        })
    return in_maps


def postprocess(results, n):
    new_h = np.concatenate(
        [np.asarray(results[c]["nhT_out"]).astype(np.float32).T
         for c in range(N_CORES)], axis=0)
    par = np.asarray(results[0]["par_out"]).reshape(-1)
    S = par[HID:HID + OUT]
    Z = par[192]
    tsum = par[193]
    combined = (S / Z)[None, :].astype(np.float32)
    avg_tension = np.float32(tsum / n)
    return combined, avg_tension, new_h


_CACHE = {}


def kernel(x, hiddens, Wa1, ba1, Wa2, ba2, Wg1, bg1, Wg2, bg2,
           Wih, Whh, bih, bhh, step, trace=False):
    _install_ntff_hook()
    hiddens = np.asarray(hiddens, np.float32)
    n = hiddens.shape[0]
    n_loc = n // N_CORES

    if n_loc not in _CACHE:
        _CACHE[n_loc] = build_kernel(n_loc)
    nc = _CACHE[n_loc]

    consts = make_host_consts(x, Wa1, ba1, Wa2, ba2, Wg1, bg1, Wg2, bg2,
                              Wih, Whh, bih, bhh, step, n_loc)
    in_maps = make_in_maps(hiddens, consts, n_loc)

    res = run_bass_kernel_spmd(nc, in_maps, list(range(N_CORES)), trace=trace)
    kernel.last_exec_time_ns = res.exec_time_ns
    return postprocess(res.results, n)


kernel.last_exec_time_ns = None


# revision 20
# speedup vs baseline: 1.1089x; 1.1089x over previous
"""Trainium2 Bass kernel for the BaselineEngine cell-swarm module.

Sharding: data-parallel over the cell axis across 8 NeuronCores; 32768
cells per core = exactly one faction per core, so faction means are
core-local and only a tiny packed AllReduce ([256] floats: faction sum,
softmax numerator S, softmax denominator Z, tension sum) crosses cores.

Per-core layout is feature-major ([feature, cell]) so that the MLP and
GRU matmuls chain on the TensorEngine without transposes.  The single
shared input x is folded into the first-layer biases on the host.
Sigmoid is computed as (1 + tanh(x/2))/2 so every transcendental lives
in the one ACT table set; the resulting affine terms are folded into
the PE-side accumulations and the GRU combine:

    new_h_pre = nctl + z*(h - nctl),  z = (1+th_z)/2
    w := 2*new_h_pre = (th_z+1)*h - (th_z-1)*nctl

w is kept resident in SBUF; after a local faction-sum reduce and the
AllReduce, phase 2 blends w with the faction/global means and streams
the result out.
"""

import sys
import types

sys.path.insert(0, "/opt/trn_rl_repo")

import numpy as np
import ml_dtypes

import concourse.bass as bass
import concourse.bacc as bacc
import concourse.tile as tile
from concourse import mybir
from concourse.bass_utils import run_bass_kernel_spmd

BF16 = mybir.dt.bfloat16
F32 = mybir.dt.float32
AL = mybir.AluOpType
AF = mybir.ActivationFunctionType

N_CORES = 8
HID = 128
OUT = 64
IN_DIM = 64
FD_G = 2048          # cells per group (SBUF-op granularity)
FD_C = 1024          # cells per PSUM chunk
MM_N = 512           # matmul moving free dim


def _install_ntff_hook():
    """Make antenv.axon_hooks importable so trace=True works here."""
    if "antenv.axon_hooks" in sys.modules:
        return
    try:
        import antenv
        from trn_agent_boot.trn_boot import _ntff_profile_via_ctypes
    except Exception:
        return
    mod = types.ModuleType("antenv.axon_hooks")
    mod._hook = None

    def set_axon_ntff_profile_hook(h):
        mod._hook = h

    def get_axon_ntff_profile_hook():
        return mod._hook

    mod.set_axon_ntff_profile_hook = set_axon_ntff_profile_hook
    mod.get_axon_ntff_profile_hook = get_axon_ntff_profile_hook
    sys.modules["antenv.axon_hooks"] = mod
    antenv.axon_hooks = mod
    try:
        h = _ntff_profile_via_ctypes("/opt/axon/libaxon_pjrt.so")
        if h is not None:
            set_axon_ntff_profile_hook(h)
    except Exception:
        pass


def build_kernel(n_loc: int, n_cores: int = N_CORES):
    """Emit the per-core Bass program for n_loc cells per core."""
    n_groups = n_loc // FD_G
    n_chunks = n_loc // FD_C
    # debate rows = first fs//4 cells of the faction; group-aligned by design
    dc = n_loc // 4
    assert dc % FD_G == 0
    dbg_groups = dc // FD_G

    nc = bacc.Bacc(None, num_devices=n_cores)

    hT_in = nc.declare_dram_parameter("hT_in", [HID, n_loc], BF16, False)
    waT_in = nc.declare_dram_parameter("waT_in", [HID, HID], BF16, False)
    wgT_in = nc.declare_dram_parameter("wgT_in", [HID, HID], BF16, False)
    w2T_in = nc.declare_dram_parameter("w2T_in", [HID, HID], BF16, False)
    wihT_in = nc.declare_dram_parameter("wihT_in", [OUT + 2, 3 * HID], BF16, False)
    whhT_in = nc.declare_dram_parameter("whhT_in", [HID, 2 * HID], BF16, False)
    whhTn_in = nc.declare_dram_parameter("whhTn_in", [HID, HID], BF16, False)
    cst_in = nc.declare_dram_parameter("cst_in", [HID, 8], F32, False)
    nhT_out = nc.declare_dram_parameter("nhT_out", [HID, n_loc], BF16, True)
    par_out = nc.declare_dram_parameter("par_out", [256, 1], F32, True)

    with tile.TileContext(nc) as tc:
        with (
            tc.tile_pool(name="const", bufs=1) as constp,
            tc.tile_pool(name="pre", bufs=1) as prep,
            tc.tile_pool(name="acc", bufs=1) as accp,
            tc.tile_pool(name="hin", bufs=4) as hinp,
            tc.tile_pool(name="work", bufs=2) as workp,
            tc.tile_pool(name="late", bufs=2) as latep,
            tc.tile_pool(name="psf", bufs=2, space="PSUM") as psfp,
            tc.tile_pool(name="psm", bufs=2, space="PSUM") as psmp,
            tc.tile_pool(name="dram", bufs=1, space="DRAM") as dramp,
        ):
            # ---- constants ----
            waT = constp.tile([HID, HID], BF16)
            wgT = constp.tile([HID, HID], BF16)
            w2T = constp.tile([HID, HID], BF16)       # [:,0:64]=Wa2T, [:,64:128]=-Wg2T
            wihT = constp.tile([OUT + 2, 3 * HID], BF16)
            whhT = constp.tile([HID, 2 * HID], BF16)
            whhTn = constp.tile([HID, HID], BF16)     # 0.5*Whh_n^T
            cst = constp.tile([HID, 8], F32)
            ones65 = constp.tile([OUT, OUT + 1], BF16)
            ones_row = constp.tile([1, FD_G], BF16)
            nc.vector.memset(ones_row[:], 1.0)
            nc.sync.dma_start(out=waT[:], in_=waT_in[:])
            nc.sync.dma_start(out=wgT[:], in_=wgT_in[:])
            nc.sync.dma_start(out=w2T[:], in_=w2T_in[:])
            nc.sync.dma_start(out=wihT[:], in_=wihT_in[:])
            nc.sync.dma_start(out=whhT[:], in_=whhT_in[:])
            nc.sync.dma_start(out=whhTn[:], in_=whhTn_in[:])
            nc.sync.dma_start(out=cst[:], in_=cst_in[:])
            nc.vector.memset(ones65[:], 1.0)
            abias = cst[:, 0:1]
            gbias = cst[:, 1:2]
            dba = cst[0:OUT, 2:3]
            bhhn_half = cst[:, 3:4]
            dalpha = cst[:, 4:5]
            dc1 = cst[:, 5:6]
            dc2 = cst[:, 6:7]

            # ---- persistent state ----
            pre = prep.tile([HID, n_loc], BF16)            # w = 2*new_h_pre
            wsum = accp.tile([HID, n_groups], F32)         # per-group sum of w
            s_cols = accp.tile([OUT, n_chunks], F32)       # softmax numerator partials
            z_cols = accp.tile([HID, n_chunks], F32)       # rows 0:65 valid
            t_cols = accp.tile([HID, n_chunks], F32)       # row 64 valid

            # =================== phase 1 (software-pipelined) ===================
            NCC = FD_G // FD_C
            tiles = {}

            def front(g):
                c0 = g * FD_G
                hT = hinp.tile([HID, FD_G], BF16, tag="hT")
                nc.sync.dma_start(out=hT[:], in_=hT_in[:, c0:c0 + FD_G])
                mem = workp.tile([OUT + 2, FD_G], BF16, tag="mem")
                nc.sync.dma_start(out=mem[OUT + 1:OUT + 2, :], in_=ones_row[:])
                h1a = workp.tile([HID, FD_G], BF16, tag="h1a")
                h1g = workp.tile([HID, FD_G], BF16, tag="h1g")
                out2 = workp.tile([OUT, FD_G], BF16, tag="o2w")
                wexp = workp.tile([OUT + 1, FD_G], BF16, tag="wexp")
                wsc = workp.tile([OUT, FD_G], BF16, tag="o2w")
                tiles[g] = dict(hT=hT, mem=mem)
                for cc in range(NCC):
                    sl = slice(cc * FD_C, (cc + 1) * FD_C)
                    ci = g * NCC + cc
                    psA = psfp.tile([HID, FD_C], F32, tag="psf")
                    for q in range(FD_C // MM_N):
                        qs = slice(cc * FD_C + q * MM_N, cc * FD_C + (q + 1) * MM_N)
                        qd = slice(q * MM_N, (q + 1) * MM_N)
                        nc.tensor.matmul(psA[:, qd], waT[:], hT[:, qs], start=True, stop=True)
                    nc.scalar.activation(out=h1a[:, sl], in_=psA[:], func=AF.Relu,
                                         bias=abias, scale=1.0)
                    psG = psfp.tile([HID, FD_C], F32, tag="psf")
                    for q in range(FD_C // MM_N):
                        qs = slice(cc * FD_C + q * MM_N, cc * FD_C + (q + 1) * MM_N)
                        qd = slice(q * MM_N, (q + 1) * MM_N)
                        nc.tensor.matmul(psG[:, qd], wgT[:], hT[:, qs], start=True, stop=True)
                    nc.scalar.activation(out=h1g[:, sl], in_=psG[:], func=AF.Relu,
                                         bias=gbias, scale=1.0)
                    psO = psfp.tile([HID, FD_C], F32, tag="psf")
                    for q in range(FD_C // MM_N):
                        qs = slice(cc * FD_C + q * MM_N, cc * FD_C + (q + 1) * MM_N)
                        qd = slice(q * MM_N, (q + 1) * MM_N)
                        nc.tensor.matmul(psO[0:OUT, qd], w2T[:, 0:OUT], h1a[:, qs], start=True, stop=False)
                        nc.tensor.matmul(psO[0:OUT, qd], w2T[:, OUT:HID], h1g[:, qs], start=False, stop=True)
                    nc.scalar.activation(out=mem[0:OUT, sl], in_=psO[0:OUT, :],
                                          func=AF.Identity, bias=dba, scale=1.0)
                    nc.vector.tensor_tensor(out=out2[:, sl], in0=mem[0:OUT, sl],
                                            in1=mem[0:OUT, sl], op=AL.mult)
                    psT = psfp.tile([HID, FD_C], F32, tag="psf")
                    for q in range(FD_C // MM_N):
                        qs = slice(cc * FD_C + q * MM_N, cc * FD_C + (q + 1) * MM_N)
                        qd = slice(q * MM_N, (q + 1) * MM_N)
                        nc.tensor.matmul(psT[0:OUT + 1, qd], ones65[:], out2[:, qs], start=True, stop=True)
                    nc.scalar.activation(out=wexp[:, sl], in_=psT[0:OUT + 1, :], func=AF.Exp,
                                         scale=1.0 / OUT, accum_out=z_cols[0:OUT + 1, ci:ci + 1])
                    nc.scalar.activation(out=mem[OUT:OUT + 1, sl], in_=psT[OUT:OUT + 1, :],
                                         func=AF.Copy, scale=1.0 / OUT,
                                         accum_out=t_cols[OUT:OUT + 1, ci:ci + 1])
                    nc.vector.scalar_tensor_tensor(
                        out=wsc[:, sl], in0=mem[0:OUT, sl], scalar=0.0, in1=wexp[0:OUT, sl],
                        op0=AL.bypass, op1=AL.mult, accum_out=s_cols[:, ci:ci + 1])

            def mid(g):
                d = tiles[g]
                hT, mem = d["hT"], d["mem"]
                thr = workp.tile([HID, FD_G], BF16, tag="thr")
                thz = workp.tile([HID, FD_G], BF16, tag="thz")
                mtl = latep.tile([HID, FD_G], BF16, tag="mtl")
                targ = latep.tile([HID, FD_G], BF16, tag="targ")
                d.update(thr=thr, thz=thz, targ=targ)
                for cc in range(NCC):
                    sl = slice(cc * FD_C, (cc + 1) * FD_C)
                    psR = psmp.tile([HID, FD_C], F32, tag="psm")
                    for q in range(FD_C // MM_N):
                        qs = slice(cc * FD_C + q * MM_N, cc * FD_C + (q + 1) * MM_N)
                        qd = slice(q * MM_N, (q + 1) * MM_N)
                        nc.tensor.matmul(psR[:, qd], wihT[:, 0:HID], mem[:, qs], start=True, stop=False)
                        nc.tensor.matmul(psR[:, qd], whhT[:, 0:HID], hT[:, qs], start=False, stop=True)
                    nc.scalar.activation(out=thr[:, sl], in_=psR[:], func=AF.Tanh, scale=0.5)
                    psZ = psmp.tile([HID, FD_C], F32, tag="psm")
                    for q in range(FD_C // MM_N):
                        qs = slice(cc * FD_C + q * MM_N, cc * FD_C + (q + 1) * MM_N)
                        qd = slice(q * MM_N, (q + 1) * MM_N)
                        nc.tensor.matmul(psZ[:, qd], wihT[:, HID:2 * HID], mem[:, qs], start=True, stop=False)
                        nc.tensor.matmul(psZ[:, qd], whhT[:, HID:2 * HID], hT[:, qs], start=False, stop=True)
                    nc.scalar.activation(out=thz[:, sl], in_=psZ[:], func=AF.Tanh, scale=0.5)
                    psA2 = psmp.tile([HID, FD_C], F32, tag="psm")
                    for q in range(FD_C // MM_N):
                        qs = slice(cc * FD_C + q * MM_N, cc * FD_C + (q + 1) * MM_N)
                        qd = slice(q * MM_N, (q + 1) * MM_N)
                        nc.tensor.matmul(psA2[:, qd], wihT[:, 2 * HID:3 * HID], mem[:, qs], start=True, stop=False)
                        nc.tensor.matmul(psA2[:, qd], whhTn[:], hT[:, qs], start=False, stop=True)
                    psB2 = psmp.tile([HID, FD_C], F32, tag="psm")
                    for q in range(FD_C // MM_N):
                        qs = slice(cc * FD_C + q * MM_N, cc * FD_C + (q + 1) * MM_N)
                        qd = slice(q * MM_N, (q + 1) * MM_N)
                        nc.tensor.matmul(psB2[:, qd], whhTn[:], hT[:, qs], start=True, stop=True)
                    nc.vector.scalar_tensor_tensor(
                        out=mtl[:, sl], in0=psB2[:], scalar=bhhn_half, in1=thr[:, sl],
                        op0=AL.add, op1=AL.mult)
                    nc.vector.tensor_tensor(out=targ[:, sl], in0=mtl[:, sl],
                                            in1=psA2[:], op=AL.add)

            def back(g):
                d = tiles.pop(g)
                c0 = g * FD_G
                nctl = latep.tile([HID, FD_G], BF16, tag="nctl")
                u1 = latep.tile([HID, FD_G], BF16, tag="u1")
                u2 = latep.tile([HID, FD_G], BF16, tag="mtl")
                nc.scalar.activation(out=nctl[:], in_=d["targ"][:], func=AF.Tanh)
                nc.vector.scalar_tensor_tensor(out=u1[:], in0=d["thz"][:], scalar=1.0,
                                               in1=d["hT"][:], op0=AL.add, op1=AL.mult)
                nc.vector.scalar_tensor_tensor(out=u2[:], in0=d["thz"][:], scalar=-1.0,
                                               in1=nctl[:], op0=AL.add, op1=AL.mult)
                nc.vector.scalar_tensor_tensor(
                    out=pre[:, c0:c0 + FD_G], in0=u1[:], scalar=0.0, in1=u2[:],
                    op0=AL.bypass, op1=AL.subtract, accum_out=wsum[:, g:g + 1])

            for t in range(n_groups + 2):
                if t < n_groups:
                    front(t)
                if 1 <= t <= n_groups:
                    mid(t - 1)
                if t >= 2:
                    back(t - 2)

            # =================== reductions + collective ===================
            fin = accp.tile([HID, 4], F32)
            nc.vector.tensor_reduce(out=fin[:, 0:1], in_=wsum[:], axis=mybir.AxisListType.X, op=AL.add)
            nc.vector.tensor_reduce(out=fin[0:OUT, 1:2], in_=s_cols[:], axis=mybir.AxisListType.X, op=AL.add)
            nc.vector.tensor_reduce(out=fin[0:1, 2:3], in_=z_cols[0:1, :], axis=mybir.AxisListType.X, op=AL.add)
            nc.vector.tensor_reduce(out=fin[OUT:OUT + 1, 3:4], in_=t_cols[OUT:OUT + 1, :],
                                    axis=mybir.AxisListType.X, op=AL.add)
            zpad = accp.tile([62, 1], F32)
            nc.vector.memset(zpad[:], 0.0)

            cc_in = dramp.tile([256, 1], F32)
            cc_out = dramp.tile([256, 1], F32)
            nc.sync.dma_start(out=cc_in[0:HID, :], in_=fin[:, 0:1])
            nc.sync.dma_start(out=cc_in[HID:HID + OUT, :], in_=fin[0:OUT, 1:2])
            nc.sync.dma_start(out=cc_in[192:193, :], in_=fin[0:1, 2:3])
            nc.sync.dma_start(out=cc_in[193:194, :], in_=fin[OUT:OUT + 1, 3:4])
            nc.sync.dma_start(out=cc_in[194:256, :], in_=zpad[:])
            nc.gpsimd.collective_compute(
                "AllReduce", AL.add, replica_groups=[list(range(n_cores))],
                ins=[cc_in.opt()], outs=[cc_out.opt()])
            nc.sync.dma_start(out=par_out[:], in_=cc_out[:])

            gsum = accp.tile([HID, 2], F32)
            nc.sync.dma_start(out=gsum[:, 0:1], in_=cc_out[0:HID, :])

            # =================== phase 2: blends ===================
            # non-debate: nh = 0.425*w + (0.075/n_loc)*fsum_w
            bv = accp.tile([HID, 2], F32)
            nc.vector.tensor_scalar(out=bv[:, 0:1], in0=fin[:, 0:1],
                                    scalar1=0.075 / n_loc, scalar2=None, op0=AL.mult)
            # debate: nh = dalpha*w + dc1*fsum_w + dc2*gsum_w
            nc.vector.tensor_scalar(out=gsum[:, 1:2], in0=fin[:, 0:1],
                                    scalar1=dc1, scalar2=None, op0=AL.mult)
            nc.vector.scalar_tensor_tensor(out=bv[:, 1:2], in0=gsum[:, 0:1],
                                           scalar=dc2, in1=gsum[:, 1:2],
                                           op0=AL.mult, op1=AL.add)

            dbg_p2 = dbg_groups
            order = list(range(dbg_p2, n_groups)) + list(range(dbg_p2))
            _tags = ["u1", "nctl", "mtl", "targ", "thr", "thz"]
            for i, g in enumerate(order):
                c0 = g * FD_G
                pool = latep if _tags[i % 6] not in ("thr", "thz") else workp
                st = pool.tile([HID, FD_G], BF16, tag=_tags[i % 6])
                if g < dbg_p2:
                    nc.vector.tensor_scalar(out=st[:], in0=pre[:, c0:c0 + FD_G],
                                            scalar1=dalpha, scalar2=bv[:, 1:2],
                                            op0=AL.mult, op1=AL.add)
                else:
                    nc.vector.tensor_scalar(out=st[:], in0=pre[:, c0:c0 + FD_G],
                                            scalar1=0.425, scalar2=bv[:, 0:1],
                                            op0=AL.mult, op1=AL.add)
                if i % 2 == 0:
                    nc.sync.dma_start(out=nhT_out[:, c0:c0 + FD_G], in_=st[:])
                else:
                    nc.gpsimd.dma_start(out=nhT_out[:, c0:c0 + FD_G], in_=st[:])

    nc.finalize()
    return nc


def make_host_consts(x, Wa1, ba1, Wa2, ba2, Wg1, bg1, Wg2, bg2,
                     Wih, Whh, bih, bhh, step, n_loc):
    """Fold x into biases; build packed bf16 weight layouts."""
    f32 = np.float32
    x0 = np.asarray(x, f32).reshape(-1)
    abias = np.asarray(ba1, f32) + np.asarray(Wa1, f32)[:, :IN_DIM] @ x0
    gbias = np.asarray(bg1, f32) + np.asarray(Wg1, f32)[:, :IN_DIM] @ x0
    dba = np.asarray(ba2, f32) - np.asarray(bg2, f32)

    bf = ml_dtypes.bfloat16
    waT = np.ascontiguousarray(np.asarray(Wa1, f32)[:, IN_DIM:].T).astype(bf)
    wgT = np.ascontiguousarray(np.asarray(Wg1, f32)[:, IN_DIM:].T).astype(bf)
    w2T = np.ascontiguousarray(
        np.concatenate([np.asarray(Wa2, f32).T, -np.asarray(Wg2, f32).T], axis=1)
    ).astype(bf)

    WihT = np.asarray(Wih, f32).T            # [65, 384]
    bih_ = np.asarray(bih, f32)
    bhh_ = np.asarray(bhh, f32)
    ones_row = np.concatenate([
        bih_[0:HID] + bhh_[0:HID],
        bih_[HID:2 * HID] + bhh_[HID:2 * HID],
        bih_[2 * HID:] + 0.5 * bhh_[2 * HID:],
    ])[None, :]
    wihT = np.ascontiguousarray(np.concatenate([WihT, ones_row], axis=0)).astype(bf)
    WhhT = np.asarray(Whh, f32).T
    whhT = np.ascontiguousarray(WhhT[:, 0:2 * HID]).astype(bf)
    whhTn = np.ascontiguousarray(0.5 * WhhT[:, 2 * HID:]).astype(bf)
    bhhn_half = 0.5 * bhh_[2 * HID:]

    # blend constants operate on w = 2*new_h_pre, hence the /2 factors
    if int(step) > 5:
        al2 = 0.85 * 0.85 / 2.0                 # debate rows: 0.7225*pre
        c1h = 0.85 * 0.15 / (2.0 * n_loc)       # 0.1275*fmean
        c2h = 0.15 / (2.0 * N_CORES * n_loc)    # 0.15*global mean
    else:
        al2 = 0.85 / 2.0
        c1h = 0.15 / (2.0 * n_loc)
        c2h = 0.0

    cst = np.zeros((HID, 8), f32)
    cst[:, 0] = abias
    cst[:, 1] = gbias
    cst[0:OUT, 2] = dba
    cst[:, 3] = bhhn_half
    cst[:, 4] = al2
    cst[:, 5] = c1h
    cst[:, 6] = c2h
    return dict(waT=waT, wgT=wgT, w2T=w2T, wihT=wihT, whhT=whhT,
                whhTn=whhTn, cst=cst)


def make_in_maps(hiddens, consts, n_loc):
    bf = ml_dtypes.bfloat16
    in_maps = []
    for c in range(N_CORES):
        shard = hiddens[c * n_loc:(c + 1) * n_loc]
        hT = np.ascontiguousarray(shard.T).astype(bf)
        in_maps.append({
            "hT_in": hT,
            "waT_in": consts["waT"], "wgT_in": consts["wgT"],
            "w2T_in": consts["w2T"], "wihT_in": consts["wihT"],
            "whhT_in": consts["whhT"], "whhTn_in": consts["whhTn"],
            "cst_in": consts["cst"],
        })
    return in_maps


def postprocess(results, n):
    new_h = np.concatenate(
        [np.asarray(results[c]["nhT_out"]).astype(np.float32).T
         for c in range(N_CORES)], axis=0)
    par = np.asarray(results[0]["par_out"]).reshape(-1)
    S = par[HID:HID + OUT]
    Z = par[192]
    tsum = par[193]
    combined = (S / Z)[None, :].astype(np.float32)
    avg_tension = np.float32(tsum / n)
    return combined, avg_tension, new_h


_CACHE = {}


def kernel(x, hiddens, Wa1, ba1, Wa2, ba2, Wg1, bg1, Wg2, bg2,
           Wih, Whh, bih, bhh, step, trace=False):
    _install_ntff_hook()
    hiddens = np.asarray(hiddens, np.float32)
    n = hiddens.shape[0]
    n_loc = n // N_CORES

    if n_loc not in _CACHE:
        _CACHE[n_loc] = build_kernel(n_loc)
    nc = _CACHE[n_loc]

    consts = make_host_consts(x, Wa1, ba1, Wa2, ba2, Wg1, bg1, Wg2, bg2,
                              Wih, Whh, bih, bhh, step, n_loc)
    in_maps = make_in_maps(hiddens, consts, n_loc)

    res = run_bass_kernel_spmd(nc, in_maps, list(range(N_CORES)), trace=trace)
    kernel.last_exec_time_ns = res.exec_time_ns
    return postprocess(res.results, n)


kernel.last_exec_time_ns = None
